# revision 3
# baseline (speedup 1.0000x reference)
"""Trainium2 Bass kernel for nn_AtomValuator (moe_routing).

Math (per state b with n=64 objects, features f=128):
  arity-1 pred p0 on rows x[b,i]            (width 128)
  arity-2 preds p1..p3 on rows [x[b,i]|x[b,j]] (width 256)
  each pred: rows + Linear2(mish(Linear1(rows)))

Key algebraic restructuring for the arity-2 preds:
  Linear1(concat(x_i, x_j)) = x_i @ W1top + x_j @ W1bot + b1 = u_i + v_j + b1
so the first layer is two tiny per-state matmuls plus a broadcast outer-sum,
instead of a [n*n, 2F] @ [2F, 2F] matmul.

mish(z) = z * tanh(ln(1 + e^z)) composed from the available ACT tables
(Exp -> Ln(w+1) -> Tanh; trn2 has no native mish/softplus tables).

The residual concat(x_i, x_j) and bias b2 are injected into the layer-2
PSUM accumulation with two extra matmuls (a 0/1 row-selector stationary and
a K=1 ones-row stationary), so the epilogue is a single DVE copy.

Sharding: core c handles states 4c..4c+3 for all of p1..p3 (data parallel)
plus rows 256c..256(c+1) of p0. All matmuls run in float32r.
"""

import sys

sys.path.insert(0, "/opt/trn_rl_repo")

import numpy as np

B, N, F = 32, 64, 128
F2 = 2 * F
NCORES = 8
SPC = B // NCORES          # states per core (pair preds)
P0R = (B * N) // NCORES    # p0 rows per core
ROWS_PS = N * N            # pair rows per state

_CACHE = {}


def _build_program():
    import concourse.bacc as bacc
    import concourse.mybir as mybir
    import concourse.tile as tile

    A = mybir.ActivationFunctionType
    dt = mybir.dt
    ALU = mybir.AluOpType

    nc = bacc.Bacc("TRN2", target_bir_lowering=False, debug=False,
                   num_devices=NCORES)

    # ---- DRAM I/O ----
    embT = nc.dram_tensor("embT", (F, SPC * N), dt.float32, kind="ExternalInput")
    xboth = nc.dram_tensor("xboth", (F, SPC, F2), dt.float32, kind="ExternalInput")
    pboth = nc.dram_tensor("pboth", (F, N // 2, F), dt.float32, kind="ExternalInput")
    w1s = nc.dram_tensor("w1s", (3, F, 2 * F2), dt.float32, kind="ExternalInput")
    w2s = nc.dram_tensor("w2s", (3, F, 2 * F2), dt.float32, kind="ExternalInput")
    b1c = nc.dram_tensor("b1c", (F, 3 * 2), dt.float32, kind="ExternalInput")
    b2row = nc.dram_tensor("b2row", (3, 1, F2), dt.float32, kind="ExternalInput")
    w1p0 = nc.dram_tensor("w1p0", (F, F), dt.float32, kind="ExternalInput")
    w2p0 = nc.dram_tensor("w2p0", (F, F), dt.float32, kind="ExternalInput")
    b1p0 = nc.dram_tensor("b1p0", (F, 1), dt.float32, kind="ExternalInput")
    embTp0 = nc.dram_tensor("embTp0", (F, P0R), dt.float32, kind="ExternalInput")
    xp0 = nc.dram_tensor("xp0", (F, 2, F), dt.float32, kind="ExternalInput")
    b2bcp0 = nc.dram_tensor("b2bcp0", (F, F), dt.float32, kind="ExternalInput")
    onesrow = nc.dram_tensor("onesrow", (1, F), dt.float32, kind="ExternalInput")

    outs = [nc.dram_tensor(f"out_p{p}", (SPC * ROWS_PS, F2), dt.float32,
                           kind="ExternalOutput") for p in (1, 2, 3)]
    out0 = nc.dram_tensor("out_p0", (P0R, F), dt.float32, kind="ExternalOutput")

    NH = 2048   # columns per elementwise chunk (half of a state's 4096 rows)
    MH = NH // F  # m-slices per chunk (16)

    with tile.TileContext(nc) as tc:
        with (
            tc.tile_pool(name="statics", bufs=1) as st,
            tc.tile_pool(name="uv", bufs=2) as uvp,
            tc.tile_pool(name="zp", bufs=4) as zp,
            tc.tile_pool(name="wp", bufs=4) as wp,
            tc.tile_pool(name="spp", bufs=4) as spp,
            tc.tile_pool(name="hp", bufs=4) as hp,
            tc.tile_pool(name="osb", bufs=6) as osb,
            tc.tile_pool(name="ps_uv", bufs=2, space="PSUM") as ps_uv,
            tc.tile_pool(name="ps_o", bufs=4, space="PSUM") as ps_o,
        ):
            # ---- static loads (gpsimd DMAs cast fp32 -> f32r) ----
            embT_r = st.tile([F, SPC * N], dt.float32r)
            nc.gpsimd.dma_start(embT_r[:], embT[:])
            xboth_r = st.tile([F, SPC, F2], dt.float32r)
            nc.gpsimd.dma_start(xboth_r[:], xboth[:])
            pboth_r = st.tile([F, N // 2, F], dt.float32r)
            nc.gpsimd.dma_start(pboth_r[:], pboth[:])
            w1r, w2r, b2r = [], [], []
            for p in range(3):
                t1 = st.tile([F, 2 * F2], dt.float32r, tag=f"w1r{p}")
                nc.gpsimd.dma_start(t1[:], w1s[p])
                w1r.append(t1)
                t2 = st.tile([F, 2 * F2], dt.float32r, tag=f"w2r{p}")
                nc.gpsimd.dma_start(t2[:], w2s[p])
                w2r.append(t2)
                tb = st.tile([1, F2], dt.float32r, tag=f"b2r{p}")
                nc.gpsimd.dma_start(tb[:], b2row[p])
                b2r.append(tb)
            b1t = st.tile([F, 6], dt.float32)
            nc.sync.dma_start(b1t[:], b1c[:])
            ones_r = st.tile([1, F], dt.float32r)
            nc.gpsimd.dma_start(ones_r[:], onesrow[:])
            w1p0_r = st.tile([F, F], dt.float32r)
            nc.gpsimd.dma_start(w1p0_r[:], w1p0[:])
            w2p0_r = st.tile([F, F], dt.float32r)
            nc.gpsimd.dma_start(w2p0_r[:], w2p0[:])
            b1p0_t = st.tile([F, 1], dt.float32)
            nc.sync.dma_start(b1p0_t[:], b1p0[:])
            embTp0_r = st.tile([F, P0R], dt.float32r)
            nc.gpsimd.dma_start(embTp0_r[:], embTp0[:])
            xp0_t = st.tile([F, 2, F], dt.float32)
            nc.sync.dma_start(xp0_t[:], xp0[:])
            b2bcp0_t = st.tile([F, F], dt.float32)
            nc.sync.dma_start(b2bcp0_t[:], b2bcp0[:])

            # ---- pair predicates ----
            for s in range(SPC):
                xT_s = embT_r[:, s * N:(s + 1) * N]
                for p in range(3):
                    # L1: uT/vT [128(k-half), 64(i)] for kh in {0,1}
                    psum_uv = ps_uv.tile([F, 4, N], dt.float32, tag="uv")
                    for kh in range(2):
                        nc.tensor.matmul(psum_uv[:, kh, :],
                                         w1r[p][:, kh * F:(kh + 1) * F],
                                         xT_s, start=True, stop=True)
                        nc.tensor.matmul(psum_uv[:, 2 + kh, :],
                                         w1r[p][:, F2 + kh * F:F2 + (kh + 1) * F],
                                         xT_s, start=True, stop=True)
                    ub = uvp.tile([F, 2, N], dt.float32, tag="ub")
                    vb = uvp.tile([F, 2, N], dt.float32, tag="vb")
                    for kh in range(2):
                        nc.vector.tensor_scalar_add(
                            ub[:, kh, :], psum_uv[:, kh, :],
                            b1t[:, 2 * p + kh:2 * p + kh + 1])
                        nc.vector.tensor_copy(vb[:, kh, :], psum_uv[:, 2 + kh, :])

                    # elementwise chain in chunks [128, 2048]
                    zs, sps = {}, {}
                    for ih in range(2):
                        IH = N // 2
                        for kh in range(2):
                            z = zp.tile([F, IH, N], dt.float32, tag="z")
                            in0 = ub[:, kh, ih * IH:(ih + 1) * IH]
                            in1 = vb[:, kh, :]
                            nc.vector.tensor_tensor(
                                z[:],
                                in0[:, :, None].broadcast_to((F, IH, N)),
                                in1[:, None, :].broadcast_to((F, IH, N)),
                                op=ALU.add)
                            w = wp.tile([F, IH * N], dt.float32, tag="w")
                            nc.scalar.activation(w[:], z[:].rearrange("p a b -> p (a b)"),
                                                 A.Exp)
                            sp = spp.tile([F, IH * N], dt.float32, tag="sp")
                            nc.scalar.activation(sp[:], w[:], A.Ln, bias=1.0)
                            zs[(ih, kh)] = z
                            sps[(ih, kh)] = sp
                    hs = {}
                    for ih in range(2):
                        for kh in range(2):
                            sp = sps[(ih, kh)]
                            t = wp.tile([F, NH], dt.float32, tag="w")
                            nc.scalar.activation(t[:], sp[:], A.Tanh)
                            h = hp.tile([F, NH], dt.float32r, tag="h")
                            nc.vector.tensor_tensor(
                                h[:], zs[(ih, kh)][:].rearrange("p a b -> p (a b)"),
                                t[:], op=ALU.mult)
                            hs[(ih, kh)] = h

                    # L2 + residual/bias injection, per m-slice [128 rows, 256]
                    for ih in range(2):
                        for mi in range(MH):
                            m = ih * MH + mi
                            po = ps_o.tile([F, F2], dt.float32, tag="po")
                            nc.tensor.matmul(po[:], hs[(ih, 0)][:, mi * F:(mi + 1) * F],
                                             w2r[p][:, 0:F2], start=True, stop=False)
                            nc.tensor.matmul(po[:], hs[(ih, 1)][:, mi * F:(mi + 1) * F],
                                             w2r[p][:, F2:2 * F2], start=False, stop=False)
                            nc.tensor.matmul(po[:], pboth_r[:, m, :],
                                             xboth_r[:, s, :], start=False, stop=False)
                            nc.tensor.matmul(po[:], ones_r[:], b2r[p][:],
                                             start=False, stop=True)
                            ot = osb.tile([F, F2], dt.float32, tag="ot")
                            nc.vector.tensor_copy(ot[:], po[:])
                            nc.sync.dma_start(
                                outs[p][(s * ROWS_PS + m * F):(s * ROWS_PS + (m + 1) * F), :],
                                ot[:])

            # ---- p0 (arity 1) ----
            psz0 = ps_uv.tile([F, P0R], dt.float32, tag="uv")
            nc.tensor.matmul(psz0[:], w1p0_r[:], embTp0_r[:], start=True, stop=True)
            w0 = wp.tile([F, P0R], dt.float32, tag="w")
            nc.scalar.activation(w0[:], psz0[:], A.Exp, bias=b1p0_t[:])
            sp0 = spp.tile([F, P0R], dt.float32, tag="sp")
            nc.scalar.activation(sp0[:], w0[:], A.Ln, bias=1.0)
            t0 = wp.tile([F, P0R], dt.float32, tag="w")
            nc.scalar.activation(t0[:], sp0[:], A.Tanh)
            z0 = zp.tile([F, P0R], dt.float32, tag="z0")
            nc.vector.tensor_scalar_add(z0[:], psz0[:], b1p0_t[:])
            h0 = hp.tile([F, P0R], dt.float32r, tag="h0")
            nc.vector.tensor_tensor(h0[:], z0[:], t0[:], op=ALU.mult)
            for m in range(2):
                po0 = ps_o.tile([F, F], dt.float32, tag="po")
                nc.tensor.matmul(po0[:], h0[:, m * F:(m + 1) * F], w2p0_r[:],
                                 start=True, stop=True)
                o0a = osb.tile([F, F], dt.float32, tag="o0a")
                nc.vector.tensor_tensor(o0a[:], po0[:], xp0_t[:, m, :], op=ALU.add)
                o0b = osb.tile([F, F], dt.float32, tag="o0b")
                nc.vector.tensor_tensor(o0b[:], o0a[:], b2bcp0_t[:], op=ALU.add)
                nc.sync.dma_start(out0[m * F:(m + 1) * F, :], o0b[:])

    nc.compile()
    return nc


def _get_nc():
    if "nc" not in _CACHE:
        _CACHE["nc"] = _build_program()
    return _CACHE["nc"]


def _prep_in_maps(inputs):
    emb = np.asarray(inputs["embeddings"], dtype=np.float32)
    x = emb.reshape(B, N, F)

    w1_, w2_, b1_, b2_ = {}, {}, {}, {}
    for idx, name in enumerate(["p1", "p2", "p3"]):
        w1_[idx] = np.asarray(inputs[f"W1_{name}"], dtype=np.float32)
        w2_[idx] = np.asarray(inputs[f"W2_{name}"], dtype=np.float32)
        b1_[idx] = np.asarray(inputs[f"b1_{name}"], dtype=np.float32)
        b2_[idx] = np.asarray(inputs[f"b2_{name}"], dtype=np.float32)

    # shared across cores
    w1s = np.stack([
        np.concatenate([w1_[p][:F, :], w1_[p][F:, :]], axis=1) for p in range(3)
    ])  # [3, 128, 512]: cols 0:256 = W1top, 256:512 = W1bot
    w2s = np.stack([
        np.concatenate([w2_[p][:F, :], w2_[p][F:, :]], axis=1) for p in range(3)
    ])  # [3, 128, 512]: cols 0:256 = W2[k<128,:], 256:512 = W2[k>=128,:]
    b1c = np.stack([b1_[p].reshape(2, F) for p in range(3)])  # [3,2,128]
    b1c = b1c.transpose(2, 0, 1).reshape(F, 6).copy()         # [128, (p,kh)]
    b2row = np.stack([b2_[p][None, :] for p in range(3)])     # [3,1,256]

    # P_both selector: [128, 32, 128]
    pb = np.zeros((F, N // 2, F), dtype=np.float32)
    for m in range(N // 2):
        for r in range(F):
            i = 2 * m + r // N
            j = r % N
            pb[i, m, r] = 1.0
            pb[N + j, m, r] += 1.0
    # note: rows 0..63 select x_i into cols 0:128 via xboth top half;
    # rows 64..127 select x_j into cols 128:256 via xboth bottom half.

    w1p0 = np.asarray(inputs["W1_p0"], dtype=np.float32)
    w2p0 = np.asarray(inputs["W2_p0"], dtype=np.float32)
    b1p0 = np.asarray(inputs["b1_p0"], dtype=np.float32)[:, None]
    b2p0 = np.asarray(inputs["b2_p0"], dtype=np.float32)
    b2bcp0 = np.broadcast_to(b2p0[None, :], (F, F)).copy()

    in_maps = []
    for c in range(NCORES):
        xs = x[c * SPC:(c + 1) * SPC]          # [4, 64, 128]
        embT = xs.reshape(SPC * N, F).T.copy()  # [128, 256]
        xboth = np.zeros((F, SPC, F2), dtype=np.float32)
        for s in range(SPC):
            xboth[:N, s, :F] = xs[s]
            xboth[N:, s, F:] = xs[s]
        rows0 = emb[c * P0R:(c + 1) * P0R]      # [256, 128]
        embTp0 = rows0.T.copy()                 # [128, 256]
        xp0 = np.stack([rows0[:F], rows0[F:]], axis=1)  # wrong orient; fix below
        # xp0 tile layout [F(part)=row-within-slice, 2(m), F(feat)]
        xp0 = np.stack([rows0[:F], rows0[F:]], axis=0).transpose(1, 0, 2).copy()
        in_maps.append({
            "embT": embT, "xboth": xboth, "pboth": pb,
            "w1s": w1s, "w2s": w2s, "b1c": b1c, "b2row": b2row,
            "w1p0": w1p0, "w2p0": w2p0, "b1p0": b1p0,
            "embTp0": embTp0, "xp0": xp0, "b2bcp0": b2bcp0,
            "onesrow": np.ones((1, F), dtype=np.float32),
        })
    return in_maps


def kernel(trace=False, **inputs):
    from concourse.bass_utils import run_bass_kernel_spmd

    nc = _get_nc()
    in_maps = _prep_in_maps(inputs)
    res = run_bass_kernel_spmd(nc, in_maps, core_ids=list(range(NCORES)),
                               trace=trace)
    _CACHE["last_result"] = res

    out_p0 = np.concatenate([res.results[c]["out_p0"] for c in range(NCORES)], axis=0)
    outs = []
    for p in (1, 2, 3):
        outs.append(np.concatenate(
            [res.results[c][f"out_p{p}"] for c in range(NCORES)], axis=0))
    return (out_p0, outs[0], outs[1], outs[2])


# revision 4
# speedup vs baseline: 1.1534x; 1.1534x over previous
"""Trainium2 Bass kernel for nn_AtomValuator (moe_routing).

Math (per state b with n=64 objects, features f=128):
  arity-1 pred p0 on rows x[b,i]            (width 128)
  arity-2 preds p1..p3 on rows [x[b,i]|x[b,j]] (width 256)
  each pred: rows + Linear2(mish(Linear1(rows)))

Key algebraic restructuring for the arity-2 preds:
  Linear1(concat(x_i, x_j)) = x_i @ W1top + x_j @ W1bot + b1 = u_i + v_j + b1
so the first layer is two tiny per-state matmuls plus a broadcast outer-sum,
instead of a [n*n, 2F] @ [2F, 2F] matmul.

mish(z) = z * tanh(ln(1 + e^z)) composed from the available ACT tables
(Exp -> Ln(w+1) -> Tanh; trn2 has no native mish/softplus tables). The ACT
table sets are steered so Exp/Ln share natural_log_exp_and_others and Tanh
uses exp_and_others, with tanh batched per (state,pred) to minimize
ACT_TABLE_LOAD thrash.

The residual concat(x_i, x_j) AND the bias b2 are injected into the layer-2
PSUM accumulation with one extra matmul per out tile: a 0/1 row-selector
stationary against an rhs of x rows pre-biased with b2/2 (each selector
column sums exactly two rhs rows, so b2/2 + b2/2 = b2). The epilogue is
then a single DVE copy psum->SBUF.

Sharding: core c handles states 4c..4c+3 for p1..p3 (data parallel) plus
rows 256c..256(c+1) of p0. Layer-2 h matmuls run in bf16 by default
(L2_BF16), everything else float32r; the residual path stays float32r/fp32.
"""

import sys

sys.path.insert(0, "/opt/trn_rl_repo")

import numpy as np

B, N, F = 32, 64, 128
F2 = 2 * F
NCORES = 8
SPC = B // NCORES          # states per core (pair preds)
P0R = (B * N) // NCORES    # p0 rows per core
ROWS_PS = N * N            # pair rows per state

L2_BF16 = True             # layer-2 h@W2 in bf16 (else float32r)
HMUL_GPSIMD = True         # h = z*t on gpsimd instead of DVE

_CACHE = {}


def _build_program():
    import bass_rust as _bass_rust
    import concourse.bacc as bacc
    import concourse.mybir as mybir
    import concourse.tile as tile
    from concourse.hw_specs import get_activation_tables
    from concourse.tile import add_dep_helper

    A = mybir.ActivationFunctionType
    dt = mybir.dt
    ALU = mybir.AluOpType

    class BaccSteered(bacc.Bacc):
        """Pin Exp/Ln to natural_log_exp_and_others and Tanh to
        exp_and_others so the table-load pass doesn't alternate sets."""

        def insert_act_table_loads(self):
            has_activation = any(
                isinstance(i, mybir.InstActivation)
                for b in self.main_func.blocks
                for i in b.instructions
            )
            if not has_activation:
                return
            filt = []
            for name, s in get_activation_tables(self.m.arch).items():
                s2 = set(s)
                if name != "natural_log_exp_and_others":
                    s2 -= {A.Exp, A.Ln}
                if name != "exp_and_others":
                    s2 -= {A.Tanh}
                filt.append((name, s2))
            _bass_rust.insert_act_table_loads(self, filt)

    nc = BaccSteered("TRN2", target_bir_lowering=False, debug=False,
                     num_devices=NCORES)

    # ---- DRAM I/O ----
    embT = nc.dram_tensor("embT", (F, SPC * N), dt.float32, kind="ExternalInput")
    xboth = nc.dram_tensor("xboth", (F, 3, SPC, F2), dt.float32, kind="ExternalInput")
    pboth = nc.dram_tensor("pboth", (F, N // 2, F), dt.float32, kind="ExternalInput")
    w1s = nc.dram_tensor("w1s", (3, F, 2 * F2), dt.float32, kind="ExternalInput")
    w2s = nc.dram_tensor("w2s", (3, F, 2 * F2), dt.float32, kind="ExternalInput")
    b1c = nc.dram_tensor("b1c", (F, 3 * 2), dt.float32, kind="ExternalInput")
    w1p0 = nc.dram_tensor("w1p0", (F, F), dt.float32, kind="ExternalInput")
    w2p0 = nc.dram_tensor("w2p0", (F, F), dt.float32, kind="ExternalInput")
    b1p0 = nc.dram_tensor("b1p0", (F, 1), dt.float32, kind="ExternalInput")
    embTp0 = nc.dram_tensor("embTp0", (F, P0R), dt.float32, kind="ExternalInput")
    xp0 = nc.dram_tensor("xp0", (F, 2, F), dt.float32, kind="ExternalInput")
    b2bcp0 = nc.dram_tensor("b2bcp0", (F, F), dt.float32, kind="ExternalInput")

    outs = [nc.dram_tensor(f"out_p{p}", (SPC * ROWS_PS, F2), dt.float32,
                           kind="ExternalOutput") for p in (1, 2, 3)]
    out0 = nc.dram_tensor("out_p0", (P0R, F), dt.float32, kind="ExternalOutput")

    IH = N // 2     # i-values per elementwise chunk
    NH = IH * N     # columns per chunk (2048)
    MH = NH // F    # m-slices per chunk (16)
    l2dt = dt.bfloat16 if L2_BF16 else dt.float32r

    with tile.TileContext(nc) as tc:
        with (
            tc.tile_pool(name="statics", bufs=1) as st,
            tc.tile_pool(name="uv", bufs=2) as uvp,
            tc.tile_pool(name="zp", bufs=4) as zp,
            tc.tile_pool(name="wp", bufs=4) as wp,
            tc.tile_pool(name="spp", bufs=4) as spp,
            tc.tile_pool(name="hp", bufs=4) as hp,
            tc.tile_pool(name="osb", bufs=4) as osb,
            tc.tile_pool(name="ps_uv", bufs=2, space="PSUM") as ps_uv,
            tc.tile_pool(name="ps_o", bufs=6, space="PSUM") as ps_o,
        ):
            hmul_eng = nc.gpsimd if HMUL_GPSIMD else nc.vector

            # ---- static loads (gpsimd DMAs cast fp32 -> f32r/bf16) ----
            embT_r = st.tile([F, SPC * N], dt.float32r)
            nc.gpsimd.dma_start(embT_r[:], embT[:])
            xboth_r = st.tile([F, 3, SPC, F2], dt.float32r)
            nc.gpsimd.dma_start(xboth_r[:], xboth[:])
            pboth_r = st.tile([F, N // 2, F], dt.float32r)
            nc.gpsimd.dma_start(pboth_r[:], pboth[:])
            w1r, w2r = [], []
            for p in range(3):
                t1 = st.tile([F, 2 * F2], dt.float32r, tag=f"w1r{p}")
                nc.gpsimd.dma_start(t1[:], w1s[p])
                w1r.append(t1)
                t2 = st.tile([F, 2 * F2], l2dt, tag=f"w2r{p}")
                nc.gpsimd.dma_start(t2[:], w2s[p])
                w2r.append(t2)
            b1t = st.tile([F, 6], dt.float32)
            nc.sync.dma_start(b1t[:], b1c[:])
            w1p0_r = st.tile([F, F], dt.float32r)
            nc.gpsimd.dma_start(w1p0_r[:], w1p0[:])
            w2p0_r = st.tile([F, F], l2dt)
            nc.gpsimd.dma_start(w2p0_r[:], w2p0[:])
            b1p0_t = st.tile([F, 1], dt.float32)
            nc.sync.dma_start(b1p0_t[:], b1p0[:])
            embTp0_r = st.tile([F, P0R], dt.float32r)
            nc.gpsimd.dma_start(embTp0_r[:], embTp0[:])
            xp0_t = st.tile([F, 2, F], dt.float32)
            nc.sync.dma_start(xp0_t[:], xp0[:])
            b2bcp0_t = st.tile([F, F], dt.float32)
            nc.sync.dma_start(b2bcp0_t[:], b2bcp0[:])

            # ---- pair predicates ----
            for s in range(SPC):
                xT_s = embT_r[:, s * N:(s + 1) * N]
                for p in range(3):
                    # L1: uT/vT [128(k-half), 64(i)] for kh in {0,1}
                    psum_uv = ps_uv.tile([F, 4, N], dt.float32, tag="uv")
                    for kh in range(2):
                        nc.tensor.matmul(psum_uv[:, kh, :],
                                         w1r[p][:, kh * F:(kh + 1) * F],
                                         xT_s, start=True, stop=True)
                        nc.tensor.matmul(psum_uv[:, 2 + kh, :],
                                         w1r[p][:, F2 + kh * F:F2 + (kh + 1) * F],
                                         xT_s, start=True, stop=True)
                    ub = uvp.tile([F, 2, N], dt.float32, tag="ub")
                    vb = uvp.tile([F, 2, N], dt.float32, tag="vb")
                    for kh in range(2):
                        nc.vector.tensor_scalar_add(
                            ub[:, kh, :], psum_uv[:, kh, :],
                            b1t[:, 2 * p + kh:2 * p + kh + 1])
                        nc.vector.tensor_copy(vb[:, kh, :], psum_uv[:, 2 + kh, :])

                    # phase A: z -> w=exp(z) -> sp=ln(1+w), 4 chunks
                    zs, sps = {}, {}
                    last_ln = None
                    for ih in range(2):
                        for kh in range(2):
                            z = zp.tile([F, IH, N], dt.float32, tag="z")
                            in0 = ub[:, kh, ih * IH:(ih + 1) * IH]
                            in1 = vb[:, kh, :]
                            nc.vector.tensor_tensor(
                                z[:],
                                in0[:, :, None].broadcast_to((F, IH, N)),
                                in1[:, None, :].broadcast_to((F, IH, N)),
                                op=ALU.add)
                            w = wp.tile([F, NH], dt.float32, tag="w")
                            nc.scalar.activation(
                                w[:], z[:].rearrange("p a b -> p (a b)"), A.Exp)
                            sp = spp.tile([F, NH], dt.float32, tag="sp")
                            last_ln = nc.scalar.activation(sp[:], w[:], A.Ln,
                                                           bias=1.0)
                            zs[(ih, kh)] = z
                            sps[(ih, kh)] = sp

                    # phase B: t=tanh(sp); h = z*t  (tanh batched per (s,p))
                    hs = {}
                    for ih in range(2):
                        for kh in range(2):
                            sp = sps[(ih, kh)]
                            t = wp.tile([F, NH], dt.float32, tag="w")
                            th = nc.scalar.activation(t[:], sp[:], A.Tanh)
                            add_dep_helper(th.ins, last_ln.ins, sync=False)
                            h = hp.tile([F, NH], l2dt, tag="h")
                            hmul_eng.tensor_tensor(
                                h[:], zs[(ih, kh)][:].rearrange("p a b -> p (a b)"),
                                t[:], op=ALU.mult)
                            hs[(ih, kh)] = h

                    # L2 + residual/b2 injection; 2 m-slices per out tile
                    for ih in range(2):
                        for mp in range(MH // 2):
                            ot = osb.tile([F, 2, F2], dt.float32, tag="ot")
                            for half in range(2):
                                mi = 2 * mp + half
                                m = ih * MH + mi
                                po = ps_o.tile([F, F2], dt.float32, tag="po")
                                nc.tensor.matmul(
                                    po[:], hs[(ih, 0)][:, mi * F:(mi + 1) * F],
                                    w2r[p][:, 0:F2], start=True, stop=False)
                                nc.tensor.matmul(
                                    po[:], hs[(ih, 1)][:, mi * F:(mi + 1) * F],
                                    w2r[p][:, F2:2 * F2], start=False, stop=False)
                                nc.tensor.matmul(
                                    po[:], pboth_r[:, m, :],
                                    xboth_r[:, p, s, :], start=False, stop=True)
                                nc.vector.tensor_copy(ot[:, half, :], po[:])
                            m0 = ih * MH + 2 * mp
                            dst = outs[p][(s * ROWS_PS + m0 * F):
                                          (s * ROWS_PS + (m0 + 2) * F), :]
                            nc.sync.dma_start(
                                dst.rearrange("(a r) n -> r a n", a=2), ot[:])

            # ---- p0 (arity 1) ----
            psz0 = ps_uv.tile([F, P0R], dt.float32, tag="uv")
            nc.tensor.matmul(psz0[:], w1p0_r[:], embTp0_r[:], start=True, stop=True)
            w0 = wp.tile([F, P0R], dt.float32, tag="w")
            nc.scalar.activation(w0[:], psz0[:], A.Exp, bias=b1p0_t[:])
            sp0 = spp.tile([F, P0R], dt.float32, tag="sp")
            nc.scalar.activation(sp0[:], w0[:], A.Ln, bias=1.0)
            t0 = wp.tile([F, P0R], dt.float32, tag="w")
            nc.scalar.activation(t0[:], sp0[:], A.Tanh)
            z0 = zp.tile([F, P0R], dt.float32, tag="z0")
            nc.vector.tensor_scalar_add(z0[:], psz0[:], b1p0_t[:])
            h0 = hp.tile([F, P0R], l2dt, tag="h0")
            nc.vector.tensor_tensor(h0[:], z0[:], t0[:], op=ALU.mult)
            for m in range(2):
                po0 = ps_o.tile([F, F], dt.float32, tag="po")
                nc.tensor.matmul(po0[:], h0[:, m * F:(m + 1) * F], w2p0_r[:],
                                 start=True, stop=True)
                o0a = osb.tile([F, F], dt.float32, tag="o0a")
                nc.vector.tensor_tensor(o0a[:], po0[:], xp0_t[:, m, :], op=ALU.add)
                o0b = osb.tile([F, F], dt.float32, tag="o0b")
                nc.vector.tensor_tensor(o0b[:], o0a[:], b2bcp0_t[:], op=ALU.add)
                nc.sync.dma_start(out0[m * F:(m + 1) * F, :], o0b[:])

    nc.compile()
    return nc


def _get_nc():
    if "nc" not in _CACHE:
        _CACHE["nc"] = _build_program()
    return _CACHE["nc"]


def _prep_in_maps(inputs):
    emb = np.asarray(inputs["embeddings"], dtype=np.float32)
    x = emb.reshape(B, N, F)

    w1_, w2_, b1_, b2_ = {}, {}, {}, {}
    for idx, name in enumerate(["p1", "p2", "p3"]):
        w1_[idx] = np.asarray(inputs[f"W1_{name}"], dtype=np.float32)
        w2_[idx] = np.asarray(inputs[f"W2_{name}"], dtype=np.float32)
        b1_[idx] = np.asarray(inputs[f"b1_{name}"], dtype=np.float32)
        b2_[idx] = np.asarray(inputs[f"b2_{name}"], dtype=np.float32)

    # shared across cores
    w1s = np.stack([
        np.concatenate([w1_[p][:F, :], w1_[p][F:, :]], axis=1) for p in range(3)
    ])  # [3, 128, 512]: cols 0:256 = W1top, 256:512 = W1bot
    w2s = np.stack([
        np.concatenate([w2_[p][:F, :], w2_[p][F:, :]], axis=1) for p in range(3)
    ])  # [3, 128, 512]: cols 0:256 = W2[k<128,:], 256:512 = W2[k>=128,:]
    b1c = np.stack([b1_[p].reshape(2, F) for p in range(3)])  # [3,2,128]
    b1c = b1c.transpose(2, 0, 1).reshape(F, 6).copy()         # [128, (p,kh)]

    # P_both selector: [128, 32, 128]; column (m,r) selects rows i(r), 64+j(r)
    pb = np.zeros((F, N // 2, F), dtype=np.float32)
    for m in range(N // 2):
        for r in range(F):
            i = 2 * m + r // N
            j = r % N
            pb[i, m, r] = 1.0
            pb[N + j, m, r] += 1.0

    w1p0 = np.asarray(inputs["W1_p0"], dtype=np.float32)
    w2p0 = np.asarray(inputs["W2_p0"], dtype=np.float32)
    b1p0 = np.asarray(inputs["b1_p0"], dtype=np.float32)[:, None]
    b2p0 = np.asarray(inputs["b2_p0"], dtype=np.float32)
    b2bcp0 = np.broadcast_to(b2p0[None, :], (F, F)).copy()

    in_maps = []
    for c in range(NCORES):
        xs = x[c * SPC:(c + 1) * SPC]          # [4, 64, 128]
        embT = xs.reshape(SPC * N, F).T.copy()  # [128, 256]
        xboth = np.zeros((F, 3, SPC, F2), dtype=np.float32)
        for p in range(3):
            for s in range(SPC):
                xboth[:N, p, s, :F] = xs[s]
                xboth[N:, p, s, F:] = xs[s]
            xboth[:, p, :, :] += b2_[p][None, None, :] * 0.5
        rows0 = emb[c * P0R:(c + 1) * P0R]      # [256, 128]
        embTp0 = rows0.T.copy()                 # [128, 256]
        xp0 = np.stack([rows0[:F], rows0[F:]], axis=0).transpose(1, 0, 2).copy()
        in_maps.append({
            "embT": embT, "xboth": xboth, "pboth": pb,
            "w1s": w1s, "w2s": w2s, "b1c": b1c,
            "w1p0": w1p0, "w2p0": w2p0, "b1p0": b1p0,
            "embTp0": embTp0, "xp0": xp0, "b2bcp0": b2bcp0,
        })
    return in_maps


def kernel(trace=False, **inputs):
    from concourse.bass_utils import run_bass_kernel_spmd

    nc = _get_nc()
    in_maps = _prep_in_maps(inputs)
    res = run_bass_kernel_spmd(nc, in_maps, core_ids=list(range(NCORES)),
                               trace=trace)
    _CACHE["last_result"] = res

    out_p0 = np.concatenate([res.results[c]["out_p0"] for c in range(NCORES)], axis=0)
    outs = []
    for p in (1, 2, 3):
        outs.append(np.concatenate(
            [res.results[c][f"out_p{p}"] for c in range(NCORES)], axis=0))
    return (out_p0, outs[0], outs[1], outs[2])
